# revision 5
# baseline (speedup 1.0000x reference)
"""Trainium2 Bass kernel for nn_AttentionBlock (Set-Transformer MAB block).

Reference computation (per batch b):
    Qp = Q @ Wq.T + bq ; Kp = K @ Wk.T + bk ; Vp = K @ Wv.T + bv   (4 heads of 64)
    A  = softmax(Qp Kp^T / 8)  ;  ctx = A Vp
    O  = LN0(Qp + ctx) ;  O = O + relu(O @ Wo.T + bo) ;  out = LN1(O)

Sharding: data-parallel over (batch, query-half) -> 8 independent shards,
one per NeuronCore, no collectives.  Each core sees its 1024 queries, the
full 2048 keys of its batch, and all weights.

Kernel layout choices:
  * scores are computed transposed (keys on partitions, ST[k,q]) so softmax
    normalization reduces over the partition dim -- done for free by an extra
    ones-column appended to V in the ctx matmul (row 64 of ctxT = colsum).
  * exp() runs on ACT with the 1/8 scale folded in; no max-subtraction
    (scores are ~N(0,1), exp can't overflow).
  * all matmuls use float32r (full-rate fp32 streaming mode).
"""

from contextlib import ExitStack

import numpy as np

import concourse.bass as bass
import concourse.tile as tile
from concourse import bacc, mybir
from concourse.bass_utils import run_bass_kernel_spmd
from concourse.masks import make_identity

FP = mybir.dt.float32
FR = mybir.dt.float32r
AF = mybir.ActivationFunctionType
OP = mybir.AluOpType

B = 4
SQ_FULL = 2048   # queries per batch
SK = 2048        # keys per batch
D = 256
H = 4
DH = D // H      # 64
NCORES = 8
QSPLIT = 2
SQ = SQ_FULL // QSPLIT    # queries per core
NQT = SQ // 128           # 8 query tiles
NKT = SK // 128           # 16 key tiles
NDT = D // 128            # 2 feature tiles
LN_EPS = 1e-5
SCALE = 0.125             # 1 / sqrt(DH)

USE_F32R = True
MT = FR if USE_F32R else FP   # dtype of matmul-feeding tiles


def R(ap):
    return ap.bitcast(FR) if USE_F32R else ap


def _emit(nc):
    Q = nc.declare_dram_parameter("Q", [SQ, D], FP, isOutput=False)
    K = nc.declare_dram_parameter("K", [SK, D], FP, isOutput=False)
    W = {
        n: nc.declare_dram_parameter(n, [D, D], FP, isOutput=False)
        for n in ("Wq", "Wk", "Wv", "Wo")
    }
    V1 = {
        n: nc.declare_dram_parameter(n, [D], FP, isOutput=False)
        for n in ("bq", "bk", "bv", "bo", "g0", "beta0", "g1", "beta1")
    }
    out = nc.declare_dram_parameter("out", [SQ, D], FP, isOutput=True)

    with tile.TileContext(nc) as tc, ExitStack() as ctx:
        singles = ctx.enter_context(tc.tile_pool(name="singles", bufs=1))
        big = ctx.enter_context(tc.tile_pool(name="big", bufs=1))
        ld = ctx.enter_context(tc.tile_pool(name="ld", bufs=6))
        ex = ctx.enter_context(tc.tile_pool(name="ex", bufs=3))
        tmp = ctx.enter_context(tc.tile_pool(name="tmp", bufs=3))
        outp = ctx.enter_context(tc.tile_pool(name="outp", bufs=3))

        ident = singles.tile([128, 128], FP)
        make_identity(nc, ident)
        identF = ident[:]
        ones41 = singles.tile([128, 4, 1], FP)
        nc.vector.memset(ones41[:], 1.0)
        epst = singles.tile([128, 1], FP)
        nc.vector.memset(epst, LN_EPS)

        def bcast(name):  # [D] dram -> [128, D] sbuf, partition-stride-0 DMA
            a = V1[name][:]
            t = singles.tile([128, D], FP, tag=f"bc_{name}")
            src = bass.AP(tensor=a.tensor, offset=a.offset, ap=[[0, 128]] + list(a.ap))
            nc.gpsimd.dma_start(out=t[:], in_=src)
            return t

        bv_b = bcast("bv")
        bo_b = bcast("bo")
        g0_b = bcast("g0")
        b0_b = bcast("beta0")
        g1_b = bcast("g1")
        b1_b = bcast("beta1")

        def ppart(name):  # [D] dram -> [128, NDT] sbuf (feature-on-partition)
            t = singles.tile([128, NDT], FP, tag=f"pp_{name}")
            nc.sync.dma_start(out=t[:], in_=V1[name][:].rearrange("(t p) -> p t", p=128))
            return t

        bq_p = ppart("bq")
        bk_p = ppart("bk")

        QpT = big.tile([128, NDT, SQ], MT)
        KpT = big.tile([128, NDT, SK], MT)
        Vp = big.tile([128, NKT, H, DH + 1], MT)
        ctxT = big.tile([DH + 1, H, SQ], MT)
        O = big.tile([128, NQT, D], FP)
        recips = big.tile([128, NQT, H], FP)

        # ================= phase 0/1: loads, transposes, projections ========
        with ExitStack() as pctx:
            mm_ps = pctx.enter_context(tc.tile_pool(name="mmps", bufs=4, space="PSUM"))

            WT = {}
            for wname in ("Wq", "Wk", "Wv", "Wo"):
                raw = ld.tile([128, NDT, D], FP, tag="wraw")
                nc.sync.dma_start(out=raw[:], in_=W[wname][:, :].rearrange("(t p) d -> p t d", p=128))
                wt = big.tile([128, NDT, D], MT, tag=f"wt_{wname}")
                for s in range(NDT):
                    ps = mm_ps.tile([128, 512], FP, tag="mm")
                    for t in range(NDT):
                        nc.tensor.transpose(
                            ps[:, t * 128:(t + 1) * 128],
                            raw[:, t, s * 128:(s + 1) * 128], identF)
                    nc.vector.tensor_copy(out=wt[:, s, :], in_=ps[:, :D])
                WT[wname] = wt

            QT = big.tile([128, NDT, SQ], MT)
            for g in range(NQT // 4):
                raws = []
                for j in range(4):
                    raw = ld.tile([128, D], FP, tag="qraw")
                    i = g * 4 + j
                    nc.sync.dma_start(out=raw[:], in_=Q[i * 128:(i + 1) * 128, :])
                    raws.append(raw)
                for s in range(NDT):
                    ps = mm_ps.tile([128, 512], FP, tag="mm")
                    for j in range(4):
                        nc.tensor.transpose(
                            ps[:, j * 128:(j + 1) * 128],
                            raws[j][:, s * 128:(s + 1) * 128], identF)
                    nc.vector.tensor_copy(out=QT[:, s, g * 512:(g + 1) * 512], in_=ps[:])
            KT = big.tile([128, NDT, SK], MT)
            for g in range(NKT // 4):
                raws = []
                for j in range(4):
                    raw = ld.tile([128, D], FP, tag="kraw")
                    i = g * 4 + j
                    nc.sync.dma_start(out=raw[:], in_=K[i * 128:(i + 1) * 128, :])
                    raws.append(raw)
                for s in range(NDT):
                    ps = mm_ps.tile([128, 512], FP, tag="mm")
                    for j in range(4):
                        nc.tensor.transpose(
                            ps[:, j * 128:(j + 1) * 128],
                            raws[j][:, s * 128:(s + 1) * 128], identF)
                    nc.vector.tensor_copy(out=KT[:, s, g * 512:(g + 1) * 512], in_=ps[:])

            # projections (feature-major QpT/KpT; bias add fused into psum->sbuf)
            for dvt in range(NDT):
                for n in range(SQ // 512):
                    ps = mm_ps.tile([128, 512], FP, tag="mm")
                    for dqt in range(NDT):
                        nc.tensor.matmul(
                            ps[:],
                            WT["Wq"][:, dqt, dvt * 128:(dvt + 1) * 128],
                            QT[:, dqt, n * 512:(n + 1) * 512],
                            start=(dqt == 0), stop=(dqt == NDT - 1))
                    nc.vector.tensor_scalar_add(
                        out=QpT[:, dvt, n * 512:(n + 1) * 512], in0=ps[:],
                        scalar1=bq_p[:, dvt:dvt + 1])
                for n in range(SK // 512):
                    ps = mm_ps.tile([128, 512], FP, tag="mm")
                    for dqt in range(NDT):
                        nc.tensor.matmul(
                            ps[:],
                            WT["Wk"][:, dqt, dvt * 128:(dvt + 1) * 128],
                            KT[:, dqt, n * 512:(n + 1) * 512],
                            start=(dqt == 0), stop=(dqt == NDT - 1))
                    nc.vector.tensor_scalar_add(
                        out=KpT[:, dvt, n * 512:(n + 1) * 512], in0=ps[:],
                        scalar1=bk_p[:, dvt:dvt + 1])
            # V projection, token-major, augmented with ones column per head
            bv_v = bv_b[:, :].rearrange("p (h d) -> p h d", h=H)
            for kt in range(NKT):
                ps = mm_ps.tile([128, 512], FP, tag="mm")
                for dqt in range(NDT):
                    nc.tensor.matmul(
                        ps[:, :D],
                        KT[:, dqt, kt * 128:(kt + 1) * 128],
                        WT["Wv"][:, dqt, :],
                        start=(dqt == 0), stop=(dqt == NDT - 1))
                nc.vector.tensor_copy(out=Vp[:, kt, :, DH:DH + 1], in_=ones41[:])
                nc.vector.tensor_add(
                    out=Vp[:, kt, :, 0:DH],
                    in0=ps[:, :D].rearrange("p (h d) -> p h d", h=H),
                    in1=bv_v)

        # ================= phase 2: attention ===============================
        with ExitStack() as pctx:
            sc_ps = pctx.enter_context(tc.tile_pool(name="scps", bufs=2, space="PSUM"))
            cx_ps = pctx.enter_context(tc.tile_pool(name="cxps", bufs=2, space="PSUM"))

            for h in range(H):
                po = (h % 2) * DH
                dvt = h // 2

                def mm_s(kt):
                    sps = sc_ps.tile([128, SQ], FP, tag="sc")
                    for n in range(SQ // 512):
                        nc.tensor.matmul(
                            sps[:, n * 512:(n + 1) * 512],
                            KpT[po:po + DH, dvt, kt * 128:(kt + 1) * 128],
                            QpT[po:po + DH, dvt, n * 512:(n + 1) * 512],
                            start=True, stop=True)
                    return sps

                cps = cx_ps.tile([DH + 1, SQ], FP, tag="cx")
                sps = mm_s(0)
                for kt in range(NKT):
                    nxt = mm_s(kt + 1) if kt + 1 < NKT else None
                    e = ex.tile([128, SQ], MT, tag="ex")
                    nc.scalar.activation(out=e[:], in_=sps[:], func=AF.Exp, scale=SCALE)
                    for n in range(SQ // 512):
                        nc.tensor.matmul(
                            cps[:, n * 512:(n + 1) * 512],
                            Vp[:, kt, h, :],
                            e[:, n * 512:(n + 1) * 512],
                            start=(kt == 0), stop=(kt == NKT - 1))
                    sps = nxt
                nc.vector.tensor_copy(out=ctxT[:, h, :], in_=cps[:])

        # ================= phase 3: merge heads + residual ==================
        with ExitStack() as pctx:
            mm_ps = pctx.enter_context(tc.tile_pool(name="mmps2", bufs=4, space="PSUM"))

            for qt in range(NQT):
                p1 = mm_ps.tile([128, 512], FP, tag="mm")  # Qp token-major
                for dvt in range(NDT):
                    nc.tensor.transpose(
                        p1[:, dvt * 128:(dvt + 1) * 128],
                        QpT[:, dvt, qt * 128:(qt + 1) * 128].bitcast(FP), identF)
                p2 = mm_ps.tile([128, 512], FP, tag="mm")  # ctx heads + colsums
                for h in range(H):
                    nc.tensor.transpose(
                        p2[:, h * (DH + 1):(h + 1) * (DH + 1)],
                        ctxT[:, h, qt * 128:(qt + 1) * 128].bitcast(FP),
                        identF[:DH + 1, :DH + 1])
                nc.vector.tensor_copy(out=O[:, qt, :], in_=p1[:, :D])
                for h in range(H):
                    nc.vector.reciprocal(
                        out=recips[:, qt, h:h + 1],
                        in_=p2[:, h * (DH + 1) + DH:h * (DH + 1) + DH + 1])
                    # O = ctx/colsum + Qp  (fused: (ctx * recip) + Qp)
                    nc.vector.scalar_tensor_tensor(
                        out=O[:, qt, h * DH:(h + 1) * DH],
                        in0=p2[:, h * (DH + 1):h * (DH + 1) + DH],
                        scalar=recips[:, qt, h:h + 1],
                        in1=O[:, qt, h * DH:(h + 1) * DH],
                        op0=OP.mult, op1=OP.add)

            # ---------- layernorm helper ----------
            def layernorm(dst, src, g_b, b_b):
                st = tmp.tile([128, 6], FP, tag="st")
                mv = tmp.tile([128, 2], FP, tag="mv")
                nc.vector.bn_stats(out=st[:], in_=src)
                nc.vector.bn_aggr(out=mv[:], in_=st[:])
                sd = tmp.tile([128, 1], FP, tag="sd")
                nc.scalar.activation(out=sd[:], in_=mv[:, 1:2], func=AF.Sqrt, bias=epst[:])
                rs = tmp.tile([128, 1], FP, tag="rs")
                nc.vector.reciprocal(out=rs[:], in_=sd[:])
                nc.vector.tensor_scalar(
                    out=dst, in0=src, scalar1=mv[:, 0:1], scalar2=rs[:],
                    op0=OP.subtract, op1=OP.mult)
                nc.vector.tensor_mul(out=dst, in0=dst, in1=g_b[:])
                nc.vector.tensor_add(out=dst, in0=dst, in1=b_b[:])

            for qt in range(NQT):
                layernorm(O[:, qt, :], O[:, qt, :], g0_b, b0_b)

            # ================= phase 5: MLP + LN1 + store ===================
            OT = big.tile([128, NDT, SQ], MT)
            for dvt in range(NDT):
                for g in range(NQT // 4):
                    ps = mm_ps.tile([128, 512], FP, tag="mm")
                    for j in range(4):
                        qt = g * 4 + j
                        nc.tensor.transpose(
                            ps[:, j * 128:(j + 1) * 128],
                            O[:, qt, dvt * 128:(dvt + 1) * 128], identF)
                    nc.vector.tensor_copy(out=OT[:, dvt, g * 512:(g + 1) * 512], in_=ps[:])
            for qt in range(NQT):
                p4 = mm_ps.tile([128, 512], FP, tag="mm")
                for dvt in range(NDT):
                    nc.tensor.matmul(
                        p4[:, :D],
                        OT[:, dvt, qt * 128:(qt + 1) * 128],
                        WT["Wo"][:, dvt, :],
                        start=(dvt == 0), stop=(dvt == NDT - 1))
                t1 = tmp.tile([128, D], FP, tag="t1")
                nc.vector.tensor_add(out=t1[:], in0=p4[:, :D], in1=bo_b[:])
                nc.vector.tensor_scalar_max(out=t1[:], in0=t1[:], scalar1=0.0)
                nc.vector.tensor_add(out=O[:, qt, :], in0=O[:, qt, :], in1=t1[:])
                f = outp.tile([128, D], FP, tag="f")
                layernorm(f[:], O[:, qt, :], g1_b, b1_b)
                nc.sync.dma_start(out=out[qt * 128:(qt + 1) * 128, :], in_=f[:])

    return nc


_NC = None


def build_nc():
    global _NC
    if _NC is None:
        nc = bacc.Bacc("TRN2", target_bir_lowering=False)
        _emit(nc)
        nc.compile()
        _NC = nc
    return _NC


def shard_inputs(Q, K, Wq, bq, Wk, bk, Wv, bv, Wo, bo, g0, beta0, g1, beta1):
    shared = {
        "Wq": Wq, "bq": bq, "Wk": Wk, "bk": bk, "Wv": Wv, "bv": bv,
        "Wo": Wo, "bo": bo, "g0": g0, "beta0": beta0, "g1": g1, "beta1": beta1,
    }
    shared = {k: np.ascontiguousarray(v, dtype=np.float32) for k, v in shared.items()}
    in_maps = []
    for c in range(NCORES):
        b, half = c // QSPLIT, c % QSPLIT
        m = dict(shared)
        m["Q"] = np.ascontiguousarray(Q[b, half * SQ:(half + 1) * SQ, :], dtype=np.float32)
        m["K"] = np.ascontiguousarray(K[b], dtype=np.float32)
        in_maps.append(m)
    return in_maps


def kernel(**inputs):
    nc = build_nc()
    in_maps = shard_inputs(**inputs)
    res = run_bass_kernel_spmd(nc, in_maps, core_ids=list(range(NCORES)))
    out = np.empty((B, SQ_FULL, D), np.float32)
    for c in range(NCORES):
        b, half = c // QSPLIT, c % QSPLIT
        out[b, half * SQ:(half + 1) * SQ, :] = res.results[c]["out"]
    return out


# revision 18
# speedup vs baseline: 1.0234x; 1.0234x over previous
"""Trainium2 Bass kernel for nn_AttentionBlock (Set-Transformer MAB block).

Reference computation (per batch b):
    Qp = Q @ Wq.T + bq ; Kp = K @ Wk.T + bk ; Vp = K @ Wv.T + bv   (4 heads of 64)
    A  = softmax(Qp Kp^T / 8)  ;  ctx = A Vp
    O  = LN0(Qp + ctx) ;  O = O + relu(O @ Wo.T + bo) ;  out = LN1(O)

Sharding: data-parallel over (batch, query-half) -> 8 independent shards,
one per NeuronCore, no collectives.  Each core sees its 1024 queries, the
full 2048 keys of its batch, and all weights.  Host-side sharding also
re-lays-out the inputs (zero-FLOP transposes): Q/K/W are shipped
feature-major so the kernel needs no on-chip input transposes.

Layout / scheduling choices:
  * scores are computed transposed (keys on partitions, ST[k,q]); the
    softmax denominator comes free from a ones-column appended to V in the
    ctx matmul (row 64 of ctxT = colsum of exp scores).  No max-subtraction
    (scores ~N(0,1), exp can't overflow).
  * ACT exp (1 elem/lane/cycle) is the pacing resource.  The head phase
    reaches the first score matmul fast; remaining projection work is
    drip-fed into PE slack during the attention loop via a filler queue.
    Per-head merge overlaps the next head's exps.  The LN/MLP tail is
    split across DVE/ACT/GPSIMD.
  * matmuls use float32r (full-rate fp32 streaming, ~1.5e-4 rel precision).
"""

from contextlib import ExitStack

import numpy as np

import concourse.bass as bass
import concourse.tile as tile
from concourse import bacc, mybir
from concourse.bass_utils import run_bass_kernel_spmd
from concourse.masks import make_identity

FP = mybir.dt.float32
FR = mybir.dt.float32r
AF = mybir.ActivationFunctionType
OP = mybir.AluOpType

B = 4
SQ_FULL = 2048   # queries per batch
SK = 2048        # keys per batch
D = 256
H = 4
DH = D // H      # 64
NCORES = 8
QSPLIT = 2
SQ = SQ_FULL // QSPLIT    # queries per core
NQT = SQ // 128           # 8 query tiles
NKT = SK // 128           # 16 key tiles
NDT = D // 128            # 2 feature tiles
LN_EPS = 1e-5
SCALE = 0.125             # 1 / sqrt(DH)

MT = FR  # dtype of matmul-feeding tiles (float32r)


def _emit(nc):
    QTd = nc.declare_dram_parameter("QT", [D, SQ], MT, isOutput=False)
    KTd = nc.declare_dram_parameter("KT", [D, SK], MT, isOutput=False)
    WTd = {
        n: nc.declare_dram_parameter(n, [D, D], MT, isOutput=False)
        for n in ("WqT", "WkT", "WvT", "WoT")
    }
    V1 = {
        n: nc.declare_dram_parameter(n, [D], FP, isOutput=False)
        for n in ("bq", "bk", "bv", "bo", "g0", "beta0", "g1", "beta1")
    }
    out = nc.declare_dram_parameter("out", [SQ, D], FP, isOutput=True)

    with tile.TileContext(nc) as tc, ExitStack() as ctx:
        singles = ctx.enter_context(tc.tile_pool(name="singles", bufs=1))
        big = ctx.enter_context(tc.tile_pool(name="big", bufs=1))
        ex = ctx.enter_context(tc.tile_pool(name="ex", bufs=3))
        ctp = ctx.enter_context(tc.tile_pool(name="ctp", bufs=2))
        tmp = ctx.enter_context(tc.tile_pool(name="tmp", bufs=6))
        outp = ctx.enter_context(tc.tile_pool(name="outp", bufs=4))

        ident = singles.tile([128, 128], FP)
        nc.vector.memset(ident[:], 0.0)
        make_identity(nc, ident, nomemset=True)
        epst = singles.tile([128, 1], FP)
        nc.vector.memset(epst, LN_EPS)
        ones41 = singles.tile([128, 4, 1], FP)
        nc.vector.memset(ones41[:], 1.0)
        onesF = singles.tile([1, 128], FP)
        nc.vector.memset(onesF[:], 1.0)

        def bcast(name):  # [D] dram -> [128, D] sbuf, partition-stride-0 DMA
            a = V1[name][:]
            t = singles.tile([128, D], FP, tag=f"bc_{name}")
            src = bass.AP(tensor=a.tensor, offset=a.offset, ap=[[0, 128]] + list(a.ap))
            nc.gpsimd.dma_start(out=t[:], in_=src)
            return t

        def ppart(name):  # [D] dram -> [128, NDT] sbuf (feature-on-partition)
            t = singles.tile([128, NDT], FP, tag=f"pp_{name}")
            nc.sync.dma_start(out=t[:], in_=V1[name][:].rearrange("(t p) -> p t", p=128))
            return t

        def layernorm(dst, src, g_b, b_b, gp_engine):
            st = tmp.tile([128, 6], FP, tag="st")
            mv = tmp.tile([128, 2], FP, tag="mv")
            nc.vector.bn_stats(out=st[:], in_=src)
            nc.vector.bn_aggr(out=mv[:], in_=st[:])
            sd = tmp.tile([128, 1], FP, tag="sd")
            nc.scalar.activation(out=sd[:], in_=mv[:, 1:2], func=AF.Sqrt, bias=epst[:])
            rs = tmp.tile([128, 1], FP, tag="rs")
            nc.vector.reciprocal(out=rs[:], in_=sd[:])
            nc.vector.tensor_scalar(
                out=dst, in0=src, scalar1=mv[:, 0:1], scalar2=rs[:],
                op0=OP.subtract, op1=OP.mult)
            gp_engine.tensor_mul(out=dst, in0=dst, in1=g_b[:])
            gp_engine.tensor_add(out=dst, in0=dst, in1=b_b[:])

        QpT = big.tile([128, NDT, SQ], MT)
        KpT = big.tile([128, NDT, SK], MT)
        Vp = big.tile([128, NKT, H, DH + 1], MT)
        O = big.tile([128, NQT, D], FP)
        recips = big.tile([128, NQT, H], FP)
        KT = big.tile([128, NDT, SK], MT)
        QT = big.tile([128, NDT, SQ], MT)
        WT = {}
        for wname in ("WqT", "WkT", "WvT", "WoT"):
            wt_tile = big.tile([128, NDT, D], MT, tag=f"wt_{wname}")
            WT[wname] = wt_tile

        # ========== phase A: loads + critical-path projections ==============
        with ExitStack() as pctx:
            mm_ps = pctx.enter_context(tc.tile_pool(name="mmps", bufs=4, space="PSUM"))

            # input DMAs spread across issue engines, ordered by first use:
            # gpsimd: Wq/Wk/Wv, bv, K chunks, Wo, remaining broadcasts;
            # sync: Q chunks + per-partition biases; ACT stays free for the
            # projection bias-moves that gate the first exp
            for wname in ("WqT", "WkT", "WvT"):
                nc.gpsimd.dma_start(
                    out=WT[wname][:],
                    in_=WTd[wname][:, :].rearrange("(s p) d -> p s d", p=128))
            for c in range(2):
                nc.sync.dma_start(
                    out=QT[:, :, c * 512:(c + 1) * 512],
                    in_=QTd[:, c * 512:(c + 1) * 512].rearrange("(s p) q -> p s q", p=128))
            bq_p = ppart("bq")
            bk_p = ppart("bk")
            bv_b = bcast("bv")
            bv_v = bv_b[:, :].rearrange("p (h d) -> p h d", h=H)
            for c in range(4):
                eng = nc.gpsimd if c % 2 == 0 else nc.sync
                eng.dma_start(
                    out=KT[:, :, c * 512:(c + 1) * 512],
                    in_=KTd[:, c * 512:(c + 1) * 512].rearrange("(s p) k -> p s k", p=128))
            nc.gpsimd.dma_start(
                out=WT["WoT"][:],
                in_=WTd["WoT"][:, :].rearrange("(s p) d -> p s d", p=128))
            bq_b = bcast("bq")
            bo_b = bcast("bo")
            g0_b = bcast("g0")
            b0_b = bcast("beta0")
            g1_b = bcast("g1")
            b1_b = bcast("beta1")

            def proj_chunk(pool, dst, wt, src, bias_p, dvt, n, on_act):
                ps = pool.tile([128, 512], FP, tag=("mm" if pool is mm_ps else "fil"))
                for dqt in range(NDT):
                    nc.tensor.matmul(
                        ps[:],
                        wt[:, dqt, dvt * 128:(dvt + 1) * 128],
                        src[:, dqt, n * 512:(n + 1) * 512],
                        start=(dqt == 0), stop=(dqt == NDT - 1))
                if on_act:
                    nc.scalar.activation(
                        out=dst[:, dvt, n * 512:(n + 1) * 512], in_=ps[:],
                        func=AF.Identity, bias=bias_p[:, dvt:dvt + 1], scale=1.0)
                else:
                    nc.vector.tensor_scalar_add(
                        out=dst[:, dvt, n * 512:(n + 1) * 512], in0=ps[:],
                        scalar1=bias_p[:, dvt:dvt + 1])

            def vp_pair(kts, pool):  # V projection for a pair of key tiles
                for kt in kts:
                    ps = pool.tile([128, 512], FP, tag=("mm" if pool is mm_ps else "fil"))
                    for dqt in range(NDT):
                        nc.tensor.matmul(
                            ps[:, :D],
                            KT[:, dqt, kt * 128:(kt + 1) * 128],
                            WT["WvT"][:, dqt, :],
                            start=(dqt == 0), stop=(dqt == NDT - 1))
                    nc.vector.tensor_copy(out=Vp[:, kt, :, DH:DH + 1], in_=ones41[:])
                    nc.vector.tensor_add(
                        out=Vp[:, kt, :, 0:DH],
                        in0=ps[:, :D].rearrange("p (h d) -> p h d", h=H),
                        in1=bv_v)

            # critical path: QpT(dvt0), KpT(dvt0, keys 0..511), Vp(0..3)
            proj_chunk(mm_ps, QpT, WT["WqT"], QT, bq_p, 0, 0, True)
            proj_chunk(mm_ps, QpT, WT["WqT"], QT, bq_p, 0, 1, True)
            proj_chunk(mm_ps, KpT, WT["WkT"], KT, bk_p, 0, 0, True)
            vp_pair((0, 1), mm_ps)
            vp_pair((2, 3), mm_ps)

        # ========== phase B: attention + fillers ============================
        with ExitStack() as pctx:
            sc_ps = pctx.enter_context(tc.tile_pool(name="scps", bufs=2, space="PSUM"))
            cx_ps = pctx.enter_context(tc.tile_pool(name="cxps", bufs=1, space="PSUM"))
            aux_ps = pctx.enter_context(tc.tile_pool(name="auxps", bufs=2, space="PSUM"))

            # residual base: O = Qp token-major, via matmul from QT
            def obase_fill(qt):
                ps = aux_ps.tile([128, 512], FP, tag="fil")
                for dqt in range(NDT):
                    nc.tensor.matmul(
                        ps[:, :D],
                        QT[:, dqt, qt * 128:(qt + 1) * 128],
                        WT["WqT"][:, dqt, :],
                        start=(dqt == 0), stop=(dqt == NDT - 1))
                nc.vector.tensor_add(out=O[:, qt, :], in0=ps[:, :D], in1=bq_b[:])

            # remaining projections, drip-fed into PE slack in dependency order
            fillers = []
            for c in range(1, 4):
                fillers.append(lambda c=c: proj_chunk(
                    aux_ps, KpT, WT["WkT"], KT, bk_p, 0, c, False))
                fillers.append(lambda c=c: vp_pair((c * 4, c * 4 + 1), aux_ps))
                fillers.append(lambda c=c: vp_pair((c * 4 + 2, c * 4 + 3), aux_ps))
            for n in range(SK // 512):
                fillers.append(lambda n=n: proj_chunk(
                    aux_ps, KpT, WT["WkT"], KT, bk_p, 1, n, False))
            for n in range(SQ // 512):
                fillers.append(lambda n=n: proj_chunk(
                    aux_ps, QpT, WT["WqT"], QT, bq_p, 1, n, False))
            for qt in range(NQT):
                fillers.append(lambda qt=qt: obase_fill(qt))

            def pump(n):
                for _ in range(n):
                    if fillers:
                        fillers.pop(0)()

            for h in range(H):
                po = (h % 2) * DH
                dvt = h // 2

                def mm_s(kt):
                    sps = sc_ps.tile([128, SQ], FP, tag="sc")
                    for n in range(SQ // 512):
                        nc.tensor.matmul(
                            sps[:, n * 512:(n + 1) * 512],
                            KpT[po:po + DH, dvt, kt * 128:(kt + 1) * 128],
                            QpT[po:po + DH, dvt, n * 512:(n + 1) * 512],
                            start=True, stop=True)
                    return sps

                cps = cx_ps.tile([DH + 1, SQ], FP, tag="cx")
                sps = mm_s(0)
                for kt in range(NKT):
                    nxt = mm_s(kt + 1) if kt + 1 < NKT else None
                    e = ex.tile([128, SQ], MT, tag="ex")
                    nc.scalar.activation(out=e[:], in_=sps[:], func=AF.Exp, scale=SCALE)
                    for n in range(SQ // 512):
                        nc.tensor.matmul(
                            cps[:, n * 512:(n + 1) * 512],
                            Vp[:, kt, h, :],
                            e[:, n * 512:(n + 1) * 512],
                            start=(kt == 0), stop=(kt == NKT - 1))
                    pump(2 if h == 0 else 1)
                    sps = nxt

                # merge this head into O while the next head's exps run
                ctxTh = ctp.tile([DH + 1, SQ], FP, tag="ct")
                if h == H - 1:
                    nc.scalar.copy(out=ctxTh[:], in_=cps[:])
                else:
                    nc.vector.tensor_copy(out=ctxTh[:], in_=cps[:])
                for qt in range(NQT):
                    pmt = aux_ps.tile([128, DH + 1], FP, tag="fil")
                    nc.tensor.transpose(
                        pmt[:], ctxTh[:, qt * 128:(qt + 1) * 128],
                        ident[:DH + 1, :DH + 1])
                    nc.vector.reciprocal(
                        out=recips[:, qt, h:h + 1], in_=pmt[:, DH:DH + 1])
                    # O = ctx/colsum + Qp  (fused multiply-add)
                    nc.vector.scalar_tensor_tensor(
                        out=O[:, qt, h * DH:(h + 1) * DH],
                        in0=pmt[:, 0:DH],
                        scalar=recips[:, qt, h:h + 1],
                        in1=O[:, qt, h * DH:(h + 1) * DH],
                        op0=OP.mult, op1=OP.add)
                    if h == H - 1:
                        layernorm(O[:, qt, :], O[:, qt, :], g0_b, b0_b, nc.gpsimd)

        # ========== phase C: LN0, MLP, LN1, store ===========================
        with ExitStack() as pctx:
            mm_ps = pctx.enter_context(tc.tile_pool(name="mmps2", bufs=4, space="PSUM"))

            ones_row = singles.tile([1, 128], MT)
            nc.vector.tensor_copy(out=ones_row[:], in_=onesF[:])
            bo_row = singles.tile([1, D], MT)
            nc.vector.tensor_copy(out=bo_row[:], in_=bo_b[0:1, :])

            OT = big.tile([128, NDT, SQ], MT)
            for qt in range(NQT):
                ps = mm_ps.tile([128, 512], FP, tag="mm")
                for dvt in range(NDT):
                    nc.tensor.transpose(
                        ps[:, dvt * 128:(dvt + 1) * 128],
                        O[:, qt, dvt * 128:(dvt + 1) * 128], ident[:])
                nc.scalar.copy(
                    out=OT[:, :, qt * 128:(qt + 1) * 128],
                    in_=ps[:, :D].rearrange("p (t x) -> p t x", t=NDT))
            for qt in range(NQT):
                p4 = mm_ps.tile([128, 512], FP, tag="mm")
                for dvt in range(NDT):
                    nc.tensor.matmul(
                        p4[:, :D],
                        OT[:, dvt, qt * 128:(qt + 1) * 128],
                        WT["WoT"][:, dvt, :],
                        start=(dvt == 0), stop=False)
                nc.tensor.matmul(
                    p4[:, :D], ones_row[:], bo_row[:], start=False, stop=True)
                t1 = tmp.tile([128, D], FP, tag="t1")
                nc.scalar.activation(out=t1[:], in_=p4[:, :D], func=AF.Relu)
                nc.vector.tensor_add(out=O[:, qt, :], in0=O[:, qt, :], in1=t1[:])
                f = outp.tile([128, D], FP, tag="f")
                layernorm(f[:], O[:, qt, :], g1_b, b1_b, nc.gpsimd)
                nc.sync.dma_start(out=out[qt * 128:(qt + 1) * 128, :], in_=f[:])

    return nc


_NC = None


def build_nc():
    global _NC
    if _NC is None:
        nc = bacc.Bacc("TRN2", target_bir_lowering=False)
        _emit(nc)
        nc.compile()
        _NC = nc
    return _NC


def shard_inputs(Q, K, Wq, bq, Wk, bk, Wv, bv, Wo, bo, g0, beta0, g1, beta1):
    # host-side zero-FLOP layout transforms: ship everything feature-major
    shared = {
        "WqT": np.asarray(Wq, dtype=np.float32).T,
        "WkT": np.asarray(Wk, dtype=np.float32).T,
        "WvT": np.asarray(Wv, dtype=np.float32).T,
        "WoT": np.asarray(Wo, dtype=np.float32).T,
        "bq": bq, "bk": bk, "bv": bv, "bo": bo,
        "g0": g0, "beta0": beta0, "g1": g1, "beta1": beta1,
    }
    shared = {k: np.ascontiguousarray(v, dtype=np.float32) for k, v in shared.items()}
    in_maps = []
    for c in range(NCORES):
        b, half = c // QSPLIT, c % QSPLIT
        m = dict(shared)
        m["QT"] = np.ascontiguousarray(
            np.asarray(Q[b, half * SQ:(half + 1) * SQ, :], dtype=np.float32).T)
        m["KT"] = np.ascontiguousarray(np.asarray(K[b], dtype=np.float32).T)
        in_maps.append(m)
    return in_maps


def kernel(**inputs):
    nc = build_nc()
    in_maps = shard_inputs(**inputs)
    res = run_bass_kernel_spmd(nc, in_maps, core_ids=list(range(NCORES)))
    out = np.empty((B, SQ_FULL, D), np.float32)
    for c in range(NCORES):
        b, half = c // QSPLIT, c % QSPLIT
        out[b, half * SQ:(half + 1) * SQ, :] = res.results[c]["out"]
    return out
